# revision 1
# baseline (speedup 1.0000x reference)
"""GAT regressor (2x GATConv + mean-pool + MLP) on 8 Trainium2 cores.

Strategy (dst-sharded, aggregate-then-transform):
- Edges sorted by destination; core c owns dst nodes [c*6250, (c+1)*6250).
- Within a core, nodes are renumbered by descending in-degree so the padded
  CSR (one [128 nodes x K_t slots] tile per 128 nodes) wastes ~6% slots.
- GAT layer 1 aggregates the 16-dim inputs x (aggregation is linear, the
  128-dim transform W1 is applied after) -> per-edge gather is 80B records
  [x(16), a_s1(4)] via one indirect DMA per tile.
- Per-node logit terms a_s/a_d are folded matvecs (x @ (W1 @ att)) computed
  on-device with a group-packed K=128 matmul.
- Softmax per dst row over the padded K slots; padding points at a dummy
  table row with a_s = -1e30 so exp() kills it.
- Layer 2 gathers records [h2(32), a_s2(1)] from an all-gathered (host
  concatenated between launches) table.
- 3 SPMD launches: host work between launches is pure concat/reshape.
"""
import numpy as np

import concourse.bass as bass
import concourse.tile as ctile
from concourse import mybir
from concourse.vector_clock import ScopedClock
from concourse.bass_utils import run_bass_kernel_spmd
from concourse.masks import make_identity

F32 = mybir.dt.float32
I32 = mybir.dt.int32
AX = mybir.AxisListType
OP = mybir.AluOpType
ACT = mybir.ActivationFunctionType

N = 50000
E0 = 1_600_000
G = 100
IN = 16
H1, C1 = 4, 32
F1 = H1 * C1              # 128
C2 = 32
NEG = 0.2
NC = 8
NL = N // NC              # 6250
P = 128
NT = (NL + P - 1) // P    # 49
NLP = NT * P              # 6272
REC1 = 20                 # [x(16), a_s1(4)]
REC2 = 36                 # [h2(32), a_s2(1), pad(3)]
T2ROWS = NC * NLP + 1


# ---------------------------------------------------------------------------
# TileContext tail-drain patch: this walrus build allows only one sem wait per
# CTRL instruction; spread the kernel-tail drain waits over several drains.
def _patched_drain_and_barrier(self, tick_clock, wait_clock):
    drain_inst = self.nc.sync.drain()
    extras = [self.nc.sync.drain() for _ in range(40)]
    wait_clock.add_sem_waits(
        drain_inst.ins, ScopedClock({None: tick_clock.global_clock})
    )
    si = drain_inst.ins.sync_info
    waits = list(si.on_wait or []) if si is not None else []
    if len(waits) > 1:
        si.on_wait = waits[:1]
        for i, w in enumerate(waits[1:]):
            esi = extras[i].ins.sync_info
            if esi is None:
                extras[i].ins.sync_info = mybir.SyncInfo(on_wait=[w], on_update=[])
            else:
                esi.on_wait = [w]
    self.nc.all_engine_barrier()
    popped = self.nc._tile_sem_poison_stack.pop()
    assert popped is self._sem_poison
    self.nc.clear_and_free_semaphores(list(self.sems.allocated().values()))
    self.nc.all_engine_barrier()


ctile.TileContext._drain_and_barrier = _patched_drain_and_barrier


def fix_multiwait(nc):
    """This walrus build allows only one sem wait per instruction: hoist all
    but one wait of any instruction onto same-engine NOPs inserted before it."""
    for f in nc.m.functions:
        for bb in f.blocks:
            lst = bb.instructions
            i = 0
            while i < len(lst):
                inst = lst[i]
                si = inst.sync_info
                waits = list(si.on_wait) if si and si.on_wait else []
                if len(waits) > 1:
                    si.on_wait = waits[-1:]
                    for w in waits[:-1]:
                        nop = mybir.InstNoOp(
                            name=nc.get_next_instruction_name(), ins=[], outs=[])
                        nop.engine = inst.engine
                        nop.sync_info = mybir.SyncInfo(on_wait=[w], on_update=[])
                        nc.register_instruction(nop)
                        lst.insert(i, nop)
                        i += 1
                i += 1


def vap(t, off, dims):
    """Flat (DRAM) AP view with extra element offset and [step,count] dims."""
    a = t[:] if not isinstance(t, bass.AP) else t
    return bass.AP(tensor=a.tensor, offset=a.offset + off, ap=dims)


def svap(t, off, free_dims):
    """SBUF AP view: keeps the base AP's partition pair (partition step must
    stay the tile's free pitch), custom free [step,count] dims + elem offset."""
    a = t[:] if not isinstance(t, bass.AP) else t
    return bass.AP(tensor=a.tensor, offset=a.offset + off,
                   ap=[list(a.ap[0])] + free_dims)


# ---------------------------------------------------------------------------
# host preprocessing: pure index/layout work
def host_prep(x, edge_index, batch):
    x = np.asarray(x, np.float32)
    ei = np.asarray(edge_index).astype(np.int64)
    batch = np.asarray(batch).astype(np.int64)

    src = np.concatenate([ei[0], np.arange(N, dtype=np.int64)]).astype(np.int32)
    dst = np.concatenate([ei[1], np.arange(N, dtype=np.int64)]).astype(np.int32)
    order = np.argsort(dst, kind="stable")
    src_s, dst_s = src[order], dst[order]
    deg = np.bincount(dst_s, minlength=N)
    rowptr = np.zeros(N + 1, np.int64)
    np.cumsum(deg, out=rowptr[1:])

    perms = []
    deg_sorted_all = []
    for c in range(NC):
        lo = c * NL
        d_local = deg[lo:lo + NL]
        perm = np.argsort(-d_local, kind="stable").astype(np.int32)
        perms.append(perm)
        deg_sorted_all.append(d_local[perm])

    # global per-tile K schedule (shared program across cores)
    Ks = []
    for t in range(NT):
        k = 0
        for c in range(NC):
            seg = deg_sorted_all[c][t * P:(t + 1) * P]
            if len(seg):
                k = max(k, int(seg.max()))
        Ks.append(max(4, ((k + 3) // 4) * 4))
    L1TOT = P * sum(Ks)

    # renumber map: orig node -> T2 row
    t2row = np.empty(N + 1, np.int32)
    for c in range(NC):
        lo = c * NL
        inv = np.empty(NL, np.int32)
        inv[perms[c]] = np.arange(NL, dtype=np.int32)
        t2row[lo:lo + NL] = c * NLP + inv
    t2row[N] = NC * NLP

    idx1s, idx2s, permidxs, onehots = [], [], [], []
    for c in range(NC):
        lo = c * NL
        perm = perms[c]
        idx1 = np.empty(L1TOT, np.int32)
        off = 0
        for t in range(NT):
            K = Ks[t]
            tbl = np.full((P, K), N, np.int32)
            for p in range(P):
                l = t * P + p
                if l >= NL:
                    continue
                n0 = lo + int(perm[l])
                e0, e1 = rowptr[n0], rowptr[n0 + 1]
                tbl[p, :e1 - e0] = src_s[e0:e1]
            idx1[off:off + P * K] = tbl.ravel()
            off += P * K
        idx1s.append(idx1)
        idx2s.append(t2row[idx1])
        pidx = np.zeros((P, NT), np.int32)
        for t in range(NT):
            for p in range(P):
                l = t * P + p
                pidx[p, t] = lo + (int(perm[l]) if l < NL else 0)
        permidxs.append(pidx)
        oh = np.zeros((P, NT, G), np.float32)
        for t in range(NT):
            for p in range(P):
                l = t * P + p
                if l < NL:
                    oh[p, t, batch[lo + perm[l]]] = 1.0
        onehots.append(oh.reshape(P, NT * G))

    x_rec = np.zeros((N + 1, REC1), np.float32)
    x_rec[:N, :IN] = x
    x_rec[N, IN:IN + 4] = -1e30

    # interleaved node->(group, col) mapping: node n = 8*j + g, so that the
    # packed phase-A output column j holds nodes 8j..8j+7 and the [32, NL]
    # a_s/a_d blocks write to [N, 4] tables with 2-dim (balanceable) DMA APs.
    xT8 = np.ascontiguousarray(
        x.reshape(NL, NC, IN).transpose(1, 2, 0).reshape(P, NL))

    cnt = np.bincount(batch, minlength=G).astype(np.float32).reshape(G, 1)

    return dict(Ks=Ks, L1TOT=L1TOT, idx1s=idx1s, idx2s=idx2s,
                permidxs=permidxs, onehots=onehots, x_rec=x_rec, xT8=xT8,
                cnt=cnt)


def fold_weights(W1, att_src1, att_dst1, b1, W2, att_src2, att_dst2):
    W1 = np.asarray(W1, np.float32)
    W1r = W1.reshape(IN, H1, C1)
    Vs = np.einsum("fhc,hc->fh", W1r, np.asarray(att_src1, np.float32))
    Vd = np.einsum("fhc,hc->fh", W1r, np.asarray(att_dst1, np.float32))
    # A8 row layout: rows 0:32 = a_s (g*4+h), rows 32:64 = a_d (g*4+h) so that
    # DMA reads start at partition 0 / 32 (quadrant rule).
    A8_lhsT = np.zeros((P, 64), np.float32)
    for g in range(NC):
        A8_lhsT[g * IN:(g + 1) * IN, g * 4:(g + 1) * 4] = Vs
        A8_lhsT[g * IN:(g + 1) * IN, 32 + g * 4:32 + (g + 1) * 4] = Vd
    W1blk = np.zeros((64, F1), np.float32)
    for h in range(H1):
        W1blk[h * IN:(h + 1) * IN, h * C1:(h + 1) * C1] = W1r[:, h, :]
    att2 = np.stack([np.asarray(att_src2, np.float32).ravel(),
                     np.asarray(att_dst2, np.float32).ravel()], 1)  # [32, 2]
    return dict(A8_lhsT=A8_lhsT, W1blk=W1blk,
                b1=np.asarray(b1, np.float32).reshape(F1, 1),
                W2=np.asarray(W2, np.float32), att2=att2)


# ---------------------------------------------------------------------------
def edge_softmax_aggregate(nc, tc, pools, idx_dram, tbl_dram, a_d_view, t, K,
                           rec, nmsg, nheads, out_cb):
    """Per-tile padded-CSR gather + segment softmax + weighted aggregation.

    a_d_view: AP [128, nheads] (per-dst attention term, this tile)
    rec: record width; nmsg: message feature count (cols 0:nmsg of record);
    a_s lives at record col nmsg..nmsg+nheads-1.
    out_cb(OPS): callback receiving [128, nheads*nmsg] aggregated+normalized.
    """
    work, psum = pools["work"], pools["psum"]
    H = nheads
    it = work.tile([P, K], I32, tag="it")
    nc.sync.dma_start(out=it[:], in_=idx_dram)
    g_ = work.tile([P, K * rec], F32, tag="g")
    # HW indirect DMA consumes ONE offset per partition (per contiguous dest
    # run), so gather one k-slot (128 rows) per instruction.
    for k in range(K):
        nc.gpsimd.indirect_dma_start(
            out=g_[:, k * rec:(k + 1) * rec], out_offset=None, in_=tbl_dram,
            in_offset=bass.IndirectOffsetOnAxis(ap=it[:, k:k + 1], axis=0))

    # logits L0[p, h, k] = a_s[src] + a_d[dst]
    L0 = work.tile([P, H * K], F32, tag="L0")
    nc.vector.tensor_tensor(
        out=L0[:],
        in0=svap(g_, nmsg, [[1, H], [rec, K]]),
        in1=svap(a_d_view, 0, [[1, H], [0, K]]),
        op=OP.add)
    # leaky relu
    Lm = work.tile([P, H * K], F32, tag="Lm")
    nc.vector.tensor_scalar_mul(Lm[:], L0[:], NEG)
    nc.vector.tensor_tensor(out=Lm[:], in0=L0[:], in1=Lm[:], op=OP.max)
    # segment max / exp / denom
    m = work.tile([P, H], F32, tag="m")
    nc.vector.tensor_reduce(
        out=m[:], in_=svap(Lm, 0, [[K, H], [1, K]]),
        axis=AX.X, op=OP.max)
    S = work.tile([P, H * K], F32, tag="S")
    nc.vector.tensor_tensor(
        out=S[:], in0=Lm[:],
        in1=svap(m, 0, [[1, H], [0, K]]), op=OP.subtract)
    # clamp: pad slots carry ~-2e29 logits; HW ACT Exp tables need sane range
    nc.vector.tensor_scalar_max(S[:], S[:], -80.0)
    EX = work.tile([P, H * K], F32, tag="EX")
    nc.scalar.activation(EX[:], S[:], ACT.Exp)
    den = work.tile([P, H], F32, tag="den")
    nc.vector.tensor_reduce(
        out=den[:], in_=svap(EX, 0, [[K, H], [1, K]]),
        axis=AX.X, op=OP.add)
    dr = work.tile([P, H], F32, tag="dr")
    nc.vector.tensor_scalar_add(dr[:], den[:], 1e-16)
    nc.vector.reciprocal(dr[:], dr[:])
    # weighted aggregation: OP[p,h,f] = sum_k EX[p,h,k] * msg[p,k,f]
    prod = work.tile([P, H * K * nmsg], F32, tag="prod")
    nc.vector.tensor_tensor(
        out=prod[:],
        in0=svap(EX, 0, [[K, H], [1, K], [0, nmsg]]),
        in1=svap(g_, 0, [[0, H], [rec, K], [1, nmsg]]),
        op=OP.mult)
    agg = work.tile([P, H * nmsg], F32, tag="agg")
    nc.vector.tensor_reduce(
        out=agg[:],
        in_=svap(prod, 0, [[K * nmsg, H], [1, nmsg], [nmsg, K]]),
        axis=AX.X, op=OP.add)
    ops = work.tile([P, H * nmsg], F32, tag="ops")
    nc.vector.tensor_tensor(
        out=ops[:], in0=agg[:],
        in1=svap(dr, 0, [[1, H], [0, nmsg]]), op=OP.mult)
    out_cb(ops)


def build_launch1(Ks, reps=0):
    nc = bass.Bass()
    L1TOT = P * sum(Ks)
    xT8 = nc.declare_dram_parameter("xT8", [P, NL], F32, isOutput=False)
    A8w = nc.declare_dram_parameter("A8w", [P, 64], F32, isOutput=False)
    x_rec = nc.declare_dram_parameter("x_rec", [N + 1, REC1], F32, isOutput=False)
    W1blk_d = nc.declare_dram_parameter("W1blk", [64, F1], F32, isOutput=False)
    b1_d = nc.declare_dram_parameter("b1", [F1, 1], F32, isOutput=False)
    W2_d = nc.declare_dram_parameter("W2", [F1, C2], F32, isOutput=False)
    att2_d = nc.declare_dram_parameter("att2", [C2, 2], F32, isOutput=False)
    idx1_d = nc.declare_dram_parameter("idx1", [L1TOT], I32, isOutput=False)
    pidx_d = nc.declare_dram_parameter("pidx", [P, NT], I32, isOutput=False)
    t2part = nc.declare_dram_parameter("t2part", [NLP, REC2], F32, isOutput=True)
    a_d2out = nc.declare_dram_parameter("a_d2out", [1, NLP], F32, isOutput=True)

    T1 = nc.dram_tensor("T1", [N + 1, REC1], F32)
    astab = nc.dram_tensor("astab", [N, 4], F32)
    adtab = nc.dram_tensor("adtab", [N, 4], F32)

    with ctile.TileContext(nc) as tc:
        import contextlib
        with contextlib.ExitStack() as ctx:
            const = ctx.enter_context(tc.tile_pool(name="const", bufs=1))
            persist = ctx.enter_context(tc.tile_pool(name="persist", bufs=1))
            work = ctx.enter_context(tc.tile_pool(name="work", bufs=2))
            psum = ctx.enter_context(tc.tile_pool(name="psum", bufs=4, space="PSUM"))
            pools = dict(work=work, psum=psum)

            ident = const.tile([P, P], F32)
            make_identity(nc, ident[:])
            w1blk_s = const.tile([64, F1], F32)
            nc.sync.dma_start(out=w1blk_s[:], in_=W1blk_d[:])
            b1_s = const.tile([F1, 1], F32)
            nc.sync.dma_start(out=b1_s[:], in_=b1_d[:])
            w2_s = const.tile([F1, C2], F32)
            nc.sync.dma_start(out=w2_s[:], in_=W2_d[:])
            att2_s = const.tile([C2, 2], F32)
            nc.sync.dma_start(out=att2_s[:], in_=att2_d[:])

            _loop = tc.For_i(0, reps, 1) if reps else contextlib.nullcontext()
            with _loop:
                # ---- phase A: per-node logit terms for all N nodes ----
                xt = persist.tile([P, NL], F32)
                nc.sync.dma_start(out=xt[:], in_=xT8[:])
                a8w_s = const.tile([P, 64], F32)
                nc.sync.dma_start(out=a8w_s[:], in_=A8w[:])
                a8s = persist.tile([64, NL], F32)
                CH = 512
                for c0 in range(0, NL, CH):
                    w = min(CH, NL - c0)
                    pz = psum.tile([64, CH], F32, tag="ps")
                    nc.tensor.matmul(pz[:, :w], lhsT=a8w_s[:], rhs=xt[:, c0:c0 + w],
                                     start=True, stop=True)
                    nc.vector.tensor_copy(out=a8s[:, c0:c0 + w], in_=pz[:, :w])

                # T1 = x_rec; then overwrite a_s columns (via node-major astab).
                # a8s partition p=4g+v, col j <-> node 8j+g: astab offset 32j+p.
                nc.sync.dma_start(out=T1[:], in_=x_rec[:])
                nc.sync.dma_start(
                    out=vap(astab, 0, [[1, 32], [32, NL]]), in_=a8s[0:32, :])
                nc.sync.dma_start(
                    out=vap(adtab, 0, [[1, 32], [32, NL]]), in_=a8s[32:64, :])
                nc.sync.dma_start(
                    out=vap(T1, IN, [[REC1, N], [1, 4]]),
                    in_=vap(astab, 0, [[4, N], [1, 4]]))

                # per-dst a_d in degree-sorted order: [128, NT*4]
                pidx_s = const.tile([P, NT], I32)
                nc.sync.dma_start(out=pidx_s[:], in_=pidx_d[:])
                adS = persist.tile([P, NT * 4], F32)
                for t in range(NT):
                    nc.gpsimd.indirect_dma_start(
                        out=adS[:, t * 4:(t + 1) * 4], out_offset=None,
                        in_=adtab[:],
                        in_offset=bass.IndirectOffsetOnAxis(
                            ap=pidx_s[:, t:t + 1], axis=0))

                # ---- layer-1 edge phase ----
                h1e = persist.tile([F1, NLP], F32)
                off = 0
                for t in range(NT):
                    K = Ks[t]
                    idx_dram = vap(idx1_d, off, [[K, P], [1, K]])
                    off += P * K

                    def finish1(ops, t=t):
                        pt = psum.tile([64, P], F32, tag="ps")
                        nc.tensor.transpose(out=pt[:], in_=ops[:], identity=ident[:, :P])
                        opst = work.tile([64, P], F32, tag="opst")
                        nc.vector.tensor_copy(out=opst[:], in_=pt[:])
                        hz = psum.tile([F1, P], F32, tag="ps")
                        nc.tensor.matmul(hz[:], lhsT=w1blk_s[:], rhs=opst[:],
                                         start=True, stop=True)
                        zb = work.tile([F1, P], F32, tag="zb")
                        nc.scalar.activation(zb[:], hz[:], ACT.Identity, bias=b1_s[:])
                        tmin = work.tile([F1, P], F32, tag="tmin")
                        nc.vector.tensor_scalar_min(tmin[:], zb[:], 0.0)
                        te = work.tile([F1, P], F32, tag="te")
                        nc.scalar.activation(te[:], tmin[:], ACT.Exp)
                        trelu = work.tile([F1, P], F32, tag="trelu")
                        nc.vector.tensor_scalar_max(trelu[:], zb[:], 0.0)
                        nc.vector.scalar_tensor_tensor(
                            out=h1e[:, t * P:(t + 1) * P], in0=te[:], scalar=-1.0,
                            in1=trelu[:], op0=OP.add, op1=OP.add)

                    edge_softmax_aggregate(
                        nc, tc, pools, idx_dram, T1[:],
                        adS[:, t * 4:(t + 1) * 4], t, K, REC1, IN, H1, finish1)

                # ---- layer-2 node phase ----
                h2a = persist.tile([C2 + 1, NLP], F32)
                adrow = persist.tile([1, NLP], F32)
                for c0 in range(0, NLP, CH):
                    w = min(CH, NLP - c0)
                    pz = psum.tile([C2, CH], F32, tag="ps")
                    nc.tensor.matmul(pz[:, :w], lhsT=w2_s[:], rhs=h1e[:, c0:c0 + w],
                                     start=True, stop=True)
                    nc.vector.tensor_copy(out=h2a[0:C2, c0:c0 + w], in_=pz[:, :w])
                    pa = psum.tile([1, CH], F32, tag="ps")
                    nc.tensor.matmul(pa[:, :w], lhsT=att2_s[:, 0:1],
                                     rhs=h2a[0:C2, c0:c0 + w], start=True, stop=True)
                    nc.vector.tensor_copy(out=h2a[C2:C2 + 1, c0:c0 + w], in_=pa[:, :w])
                    pb = psum.tile([1, CH], F32, tag="ps")
                    nc.tensor.matmul(pb[:, :w], lhsT=att2_s[:, 1:2],
                                     rhs=h2a[0:C2, c0:c0 + w], start=True, stop=True)
                    nc.vector.tensor_copy(out=adrow[:, c0:c0 + w], in_=pb[:, :w])
                nc.sync.dma_start(out=a_d2out[:], in_=adrow[:])

                # ---- T2 record assembly ----
                for t in range(NT):
                    pt = psum.tile([P, C2 + 1], F32, tag="ps")
                    nc.tensor.transpose(
                        out=pt[:], in_=h2a[:, t * P:(t + 1) * P],
                        identity=ident[0:C2 + 1, 0:C2 + 1])
                    rec = work.tile([P, REC2], F32, tag="rec")
                    nc.vector.tensor_copy(out=rec[:, 0:C2 + 1], in_=pt[:])
                    nc.vector.memset(rec[:, C2 + 1:REC2], 0.0)
                    nc.sync.dma_start(out=t2part[t * P:(t + 1) * P, :], in_=rec[:])
    fix_multiwait(nc)
    return nc


def build_launch2(Ks, reps=0):
    nc = bass.Bass()
    L1TOT = P * sum(Ks)
    T2 = nc.declare_dram_parameter("T2", [T2ROWS, REC2], F32, isOutput=False)
    idx2_d = nc.declare_dram_parameter("idx2", [L1TOT], I32, isOutput=False)
    ad2_d = nc.declare_dram_parameter("ad2", [P, NT], F32, isOutput=False)
    oh_d = nc.declare_dram_parameter("onehot", [P, NT * G], F32, isOutput=False)
    b2bc_d = nc.declare_dram_parameter("b2bc", [P, C2], F32, isOutput=False)
    partial = nc.declare_dram_parameter("partial", [G, C2], F32, isOutput=True)

    with ctile.TileContext(nc) as tc:
        import contextlib
        with contextlib.ExitStack() as ctx:
            const = ctx.enter_context(tc.tile_pool(name="const", bufs=1))
            work = ctx.enter_context(tc.tile_pool(name="work", bufs=3))
            psum = ctx.enter_context(tc.tile_pool(name="psum", bufs=4, space="PSUM"))
            ppool = ctx.enter_context(tc.tile_pool(name="ppool", bufs=1, space="PSUM"))
            pools = dict(work=work, psum=psum)

            ad2_s = const.tile([P, NT], F32)
            nc.sync.dma_start(out=ad2_s[:], in_=ad2_d[:])
            oh_s = const.tile([P, NT * G], F32)
            nc.sync.dma_start(out=oh_s[:], in_=oh_d[:])
            b2bc_s = const.tile([P, C2], F32)
            nc.sync.dma_start(out=b2bc_s[:], in_=b2bc_d[:])

            _loop = tc.For_i(0, reps, 1) if reps else contextlib.nullcontext()
            with _loop:
                pooled = ppool.tile([G, C2], F32)
                off = 0
                for t in range(NT):
                    K = Ks[t]
                    idx_dram = vap(idx2_d, off, [[K, P], [1, K]])
                    off += P * K

                    def finish2(ops, t=t):
                        zb = work.tile([P, C2], F32, tag="zb2")
                        nc.vector.tensor_tensor(out=zb[:], in0=ops[:], in1=b2bc_s[:],
                                                op=OP.add)
                        tmin = work.tile([P, C2], F32, tag="tmin2")
                        nc.vector.tensor_scalar_min(tmin[:], zb[:], 0.0)
                        te = work.tile([P, C2], F32, tag="te2")
                        nc.scalar.activation(te[:], tmin[:], ACT.Exp)
                        trelu = work.tile([P, C2], F32, tag="trelu2")
                        nc.vector.tensor_scalar_max(trelu[:], zb[:], 0.0)
                        hf = work.tile([P, C2], F32, tag="hf")
                        nc.vector.scalar_tensor_tensor(
                            out=hf[:], in0=te[:], scalar=-1.0, in1=trelu[:],
                            op0=OP.add, op1=OP.add)
                        nc.tensor.matmul(
                            pooled[:], lhsT=oh_s[:, t * G:(t + 1) * G], rhs=hf[:],
                            start=(t == 0), stop=(t == NT - 1))

                    edge_softmax_aggregate(
                        nc, tc, pools, idx_dram, T2[:],
                        ad2_s[:, t:t + 1], t, K, REC2, C2, 1, finish2)

                po = const.tile([G, C2], F32)
                nc.vector.tensor_copy(out=po[:], in_=pooled[:])
                nc.sync.dma_start(out=partial[:], in_=po[:])
    fix_multiwait(nc)
    return nc


def build_launch3(reps=0):
    nc = bass.Bass()
    parts_d = nc.declare_dram_parameter("partsT", [G, NC * C2], F32, isOutput=False)
    cnt_d = nc.declare_dram_parameter("cnt", [G, 1], F32, isOutput=False)
    Wh1_d = nc.declare_dram_parameter("Wh1", [C2, 64], F32, isOutput=False)
    bh1_d = nc.declare_dram_parameter("bh1", [64, 1], F32, isOutput=False)
    Wh2_d = nc.declare_dram_parameter("Wh2", [64, 1], F32, isOutput=False)
    bh2_d = nc.declare_dram_parameter("bh2", [1, 1], F32, isOutput=False)
    out_d = nc.declare_dram_parameter("out", [1, G], F32, isOutput=True)

    with ctile.TileContext(nc) as tc:
        import contextlib
        with contextlib.ExitStack() as ctx:
            const = ctx.enter_context(tc.tile_pool(name="const", bufs=1))
            psum = ctx.enter_context(tc.tile_pool(name="psum", bufs=1, space="PSUM"))

            ident = const.tile([P, P], F32)
            make_identity(nc, ident[:])
            parts = const.tile([G, NC * C2], F32)
            nc.sync.dma_start(out=parts[:], in_=parts_d[:])
            cnt = const.tile([G, 1], F32)
            nc.sync.dma_start(out=cnt[:], in_=cnt_d[:])
            wh1 = const.tile([C2, 64], F32)
            nc.sync.dma_start(out=wh1[:], in_=Wh1_d[:])
            bh1 = const.tile([64, 1], F32)
            nc.sync.dma_start(out=bh1[:], in_=bh1_d[:])
            wh2 = const.tile([64, 1], F32)
            nc.sync.dma_start(out=wh2[:], in_=Wh2_d[:])
            bh2 = const.tile([1, 1], F32)
            nc.sync.dma_start(out=bh2[:], in_=bh2_d[:])

            _loop = tc.For_i(0, reps, 1) if reps else contextlib.nullcontext()
            with _loop:
                sums = const.tile([G, C2], F32)
                nc.vector.tensor_reduce(
                    out=sums[:], in_=svap(parts, 0, [[1, C2], [C2, NC]]),
                    axis=AX.X, op=OP.add)
                cm = const.tile([G, 1], F32)
                nc.vector.tensor_scalar_max(cm[:], cnt[:], 1.0)
                nc.vector.reciprocal(cm[:], cm[:])
                pooled = const.tile([G, C2], F32)
                nc.vector.tensor_scalar_mul(pooled[:], sums[:], cm[:])

                pt = psum.tile([C2, G], F32)
                nc.tensor.transpose(out=pt[:], in_=pooled[:], identity=ident[:G, :G])
                pooledT = const.tile([C2, G], F32)
                nc.vector.tensor_copy(out=pooledT[:], in_=pt[:])
                z1 = psum.tile([64, G], F32)
                nc.tensor.matmul(z1[:], lhsT=wh1[:], rhs=pooledT[:], start=True, stop=True)
                r1 = const.tile([64, G], F32)
                nc.scalar.activation(r1[:], z1[:], ACT.Relu, bias=bh1[:])
                z2 = psum.tile([1, G], F32)
                nc.tensor.matmul(z2[:], lhsT=wh2[:], rhs=r1[:], start=True, stop=True)
                o = const.tile([1, G], F32)
                nc.scalar.activation(o[:], z2[:], ACT.Identity, bias=bh2[:])
                nc.sync.dma_start(out=out_d[:], in_=o[:])
    fix_multiwait(nc)
    return nc


# ---------------------------------------------------------------------------
def make_inmaps(prep, fw, inputs):
    in1 = []
    for c in range(NC):
        in1.append(dict(
            xT8=prep["xT8"], A8w=fw["A8_lhsT"], x_rec=prep["x_rec"],
            W1blk=fw["W1blk"], b1=fw["b1"], W2=fw["W2"], att2=fw["att2"],
            idx1=prep["idx1s"][c], pidx=prep["permidxs"][c]))
    return in1


def kernel(x, edge_index, batch, W1, att_src1, att_dst1, b1,
           W2, att_src2, att_dst2, b2, Wh1, bh1, Wh2, bh2):
    prep = host_prep(x, edge_index, batch)
    fw = fold_weights(W1, att_src1, att_dst1, b1, W2, att_src2, att_dst2)
    Ks = prep["Ks"]
    cores = list(range(NC))

    nc1 = build_launch1(Ks)
    res1 = run_bass_kernel_spmd(nc1, make_inmaps(prep, fw, None), cores)

    # host: concat per-core tables (pure data movement)
    T2 = np.zeros((T2ROWS, REC2), np.float32)
    for c in range(NC):
        T2[c * NLP:(c + 1) * NLP] = res1.results[c]["t2part"]
    T2[NC * NLP, C2] = -1e30
    b2bc = np.broadcast_to(np.asarray(b2, np.float32).reshape(1, C2),
                           (P, C2)).copy()
    in2 = []
    for c in range(NC):
        ad2 = res1.results[c]["a_d2out"].reshape(NT, P).T.copy()
        in2.append(dict(T2=T2, idx2=prep["idx2s"][c], ad2=ad2,
                        onehot=prep["onehots"][c], b2bc=b2bc))
    nc2 = build_launch2(Ks)
    res2 = run_bass_kernel_spmd(nc2, in2, cores)

    partsT = np.stack([res2.results[c]["partial"] for c in range(NC)], 1)  # [G, NC, C2]
    partsT = partsT.reshape(G, NC * C2)
    in3 = [dict(partsT=partsT, cnt=prep["cnt"],
                Wh1=np.asarray(Wh1, np.float32),
                bh1=np.asarray(bh1, np.float32).reshape(64, 1),
                Wh2=np.asarray(Wh2, np.float32),
                bh2=np.asarray(bh2, np.float32).reshape(1, 1))
           for _ in range(NC)]
    nc3 = build_launch3()
    res3 = run_bass_kernel_spmd(nc3, in3, cores)
    return res3.results[0]["out"].reshape(G, 1).astype(np.float32)


def _wall_min(fn, n=4):
    import time
    best = 1e9
    for _ in range(n):
        t0 = time.perf_counter()
        fn()
        best = min(best, time.perf_counter() - t0)
    return best


def _null_nc():
    nc = bass.Bass()
    x = nc.declare_dram_parameter("x", [P, 64], F32, isOutput=False)
    y = nc.declare_dram_parameter("y", [P, 64], F32, isOutput=True)
    with ctile.TileContext(nc) as tc:
        with tc.tile_pool(name="sbuf", bufs=1) as pool:
            t = pool.tile([P, 64], F32)
            nc.sync.dma_start(out=t[:], in_=x[:])
            nc.sync.dma_start(out=y[:], in_=t[:])
    fix_multiwait(nc)
    return nc


def timed_run(inputs):
    """Estimate on-device exec ns: warm per-call wall minus null-kernel wall.

    The axon PJRT path exposes no NTFF profiling and the For_i loop repeat
    trick does not compile on this toolchain, so this is an upper-bound
    estimate: per-launch warm wall minus the warm wall of a trivial kernel
    (same dispatch/tunnel overhead), floored at 0.
    """
    prep = host_prep(inputs["x"], inputs["edge_index"], inputs["batch"])
    fw = fold_weights(inputs["W1"], inputs["att_src1"], inputs["att_dst1"],
                      inputs["b1"], inputs["W2"], inputs["att_src2"],
                      inputs["att_dst2"])
    Ks = prep["Ks"]
    cores = list(range(NC))
    in1 = make_inmaps(prep, fw, None)

    nc0 = _null_nc()
    im0 = [dict(x=np.zeros((P, 64), np.float32)) for _ in range(NC)]
    run_bass_kernel_spmd(nc0, im0, cores)
    t0 = _wall_min(lambda: run_bass_kernel_spmd(nc0, im0, cores), n=5)

    nc1 = build_launch1(Ks)
    res1 = run_bass_kernel_spmd(nc1, in1, cores)
    t1 = _wall_min(lambda: run_bass_kernel_spmd(nc1, in1, cores), n=5)

    T2 = np.zeros((T2ROWS, REC2), np.float32)
    for c in range(NC):
        T2[c * NLP:(c + 1) * NLP] = res1.results[c]["t2part"]
    T2[NC * NLP, C2] = -1e30
    b2bc = np.broadcast_to(np.asarray(inputs["b2"], np.float32).reshape(1, C2),
                           (P, C2)).copy()
    in2 = []
    for c in range(NC):
        ad2 = res1.results[c]["a_d2out"].reshape(NT, P).T.copy()
        in2.append(dict(T2=T2, idx2=prep["idx2s"][c], ad2=ad2,
                        onehot=prep["onehots"][c], b2bc=b2bc))
    nc2 = build_launch2(Ks)
    run_bass_kernel_spmd(nc2, in2, cores)
    t2 = _wall_min(lambda: run_bass_kernel_spmd(nc2, in2, cores), n=5)

    d1 = max(t1 - t0, 0.0)
    d2 = max(t2 - t0, 0.0)
    print(f"null wall {t0*1e3:.1f} ms; launch1 {t1*1e3:.1f} ms; "
          f"launch2 {t2*1e3:.1f} ms")
    print(f"launch1 exec est {d1*1e6:.0f} us; launch2 exec est {d2*1e6:.0f} us")
    return (d1 + d2) * 1e9



# revision 4
# speedup vs baseline: 8.8119x; 8.8119x over previous
"""GAT regressor (2x GATConv + mean-pool + MLP) on 8 Trainium2 cores.

Single-launch, dst-sharded, aggregate-then-transform strategy:
- Edges sorted by destination; core c owns dst nodes [c*6250, (c+1)*6250).
- Within a core, nodes are renumbered by descending in-degree so the padded
  CSR (one [128 nodes x K_t slots] tile per 128 nodes) wastes ~6% slots.
- x is SHARDED across cores (each uploads its 6250 rows, bf16) and
  AllGathered on device into the full feature table.
- GAT layer 1 aggregates the 16-dim inputs (aggregation is linear, the
  128-dim transform W1 is applied after) -> per-edge gather is a 40B bf16
  record [x(16), a_s1(4)] via one indirect DMA per K-slot.
- Per-node logit terms a_s/a_d are folded matvecs (x @ (W1 @ att)) computed
  on-device with a group-packed K=128 bf16 matmul.
- Softmax per dst row over the padded K slots; padding points at a dummy
  table row with a_s = -1e30 so exp() kills it.
- Layer-2 records [h2(32), a_s2(1), pad(3)] bf16 are indirect-SCATTERED back
  to original node order locally, then AllGathered into the full T2 table,
  so layer 2 reuses layer 1's gather indices verbatim.
- Graph mean-pool via on-device one-hot (iota + is_equal) matmul, partial
  [G, C2] AllReduced across cores; the tiny MLP head runs replicated.
- Host work: index/layout preprocessing + weight folding only.
"""
import numpy as np

import concourse.bass as bass
import concourse.tile as ctile
from concourse import mybir
from concourse.vector_clock import ScopedClock
from concourse.bass_utils import run_bass_kernel_spmd
from concourse.masks import make_identity

F32 = mybir.dt.float32
BF16 = mybir.dt.bfloat16
I32 = mybir.dt.int32
U16 = mybir.dt.uint16
AX = mybir.AxisListType
OP = mybir.AluOpType
ACT = mybir.ActivationFunctionType

N = 50000
E0 = 1_600_000
G = 100
IN = 16
H1, C1 = 4, 32
F1 = H1 * C1              # 128
C2 = 32
NEG = 0.2
NC = 8
NL = N // NC              # 6250
P = 128
NT = (NL + P - 1) // P    # 49
NLP = NT * P              # 6272
REC1 = 20                 # [x(16), a_s1(4)]
REC2 = 36                 # [h2(32), a_s2(1), pad(3)]
CH = 512


# ---------------------------------------------------------------------------
# TileContext tail-drain patch: this walrus build allows only one sem wait per
# CTRL instruction; spread the kernel-tail drain waits over several drains.
def _patched_drain_and_barrier(self, tick_clock, wait_clock):
    drain_inst = self.nc.sync.drain()
    extras = [self.nc.sync.drain() for _ in range(40)]
    wait_clock.add_sem_waits(
        drain_inst.ins, ScopedClock({None: tick_clock.global_clock})
    )
    si = drain_inst.ins.sync_info
    waits = list(si.on_wait or []) if si is not None else []
    if len(waits) > 1:
        si.on_wait = waits[:1]
        for i, w in enumerate(waits[1:]):
            esi = extras[i].ins.sync_info
            if esi is None:
                extras[i].ins.sync_info = mybir.SyncInfo(on_wait=[w], on_update=[])
            else:
                esi.on_wait = [w]
    self.nc.all_engine_barrier()
    popped = self.nc._tile_sem_poison_stack.pop()
    assert popped is self._sem_poison
    self.nc.clear_and_free_semaphores(list(self.sems.allocated().values()))
    self.nc.all_engine_barrier()


ctile.TileContext._drain_and_barrier = _patched_drain_and_barrier


def fix_multiwait(nc):
    """This walrus build allows only one sem wait per instruction: hoist all
    but one wait of any instruction onto same-engine NOPs inserted before it."""
    for f in nc.m.functions:
        for bb in f.blocks:
            lst = bb.instructions
            i = 0
            while i < len(lst):
                inst = lst[i]
                si = inst.sync_info
                waits = list(si.on_wait) if si and si.on_wait else []
                if len(waits) > 1:
                    si.on_wait = waits[-1:]
                    for w in waits[:-1]:
                        nop = mybir.InstNoOp(
                            name=nc.get_next_instruction_name(), ins=[], outs=[])
                        nop.engine = inst.engine
                        nop.sync_info = mybir.SyncInfo(on_wait=[w], on_update=[])
                        nc.register_instruction(nop)
                        lst.insert(i, nop)
                        i += 1
                i += 1


def vap(t, off, dims):
    """Flat (DRAM) AP view with extra element offset and [step,count] dims."""
    a = t[:] if not isinstance(t, bass.AP) else t
    return bass.AP(tensor=a.tensor, offset=a.offset + off, ap=dims)


def svap(t, off, free_dims):
    """SBUF AP view: keeps the base AP's partition pair (partition step must
    stay the tile's free pitch), custom free [step,count] dims + elem offset."""
    a = t[:] if not isinstance(t, bass.AP) else t
    return bass.AP(tensor=a.tensor, offset=a.offset + off,
                   ap=[list(a.ap[0])] + free_dims)


# ---------------------------------------------------------------------------
# host preprocessing: pure index/layout work
def host_prep(x, edge_index, batch):
    import ml_dtypes
    x = np.asarray(x, np.float32)
    ei = np.asarray(edge_index).astype(np.int64)
    batch = np.asarray(batch).astype(np.int64)

    src = np.concatenate([ei[0], np.arange(N, dtype=np.int64)]).astype(np.int32)
    dst = np.concatenate([ei[1], np.arange(N, dtype=np.int64)]).astype(np.int32)
    order = np.argsort(dst, kind="stable")
    src_s, dst_s = src[order], dst[order]
    deg = np.bincount(dst_s, minlength=N)
    rowptr = np.zeros(N + 1, np.int64)
    np.cumsum(deg, out=rowptr[1:])

    perms = []
    deg_sorted_all = []
    for c in range(NC):
        lo = c * NL
        d_local = deg[lo:lo + NL]
        perm = np.argsort(-d_local, kind="stable").astype(np.int32)
        perms.append(perm)
        deg_sorted_all.append(d_local[perm])

    # global per-tile K schedule (shared program across cores)
    Ks = []
    for t in range(NT):
        k = 0
        for c in range(NC):
            seg = deg_sorted_all[c][t * P:(t + 1) * P]
            if len(seg):
                k = max(k, int(seg.max()))
        Ks.append(max(4, ((k + 3) // 4) * 4))
    L1TOT = P * sum(Ks)

    idx1s, permscs, pidxs, btfs = [], [], [], []
    for c in range(NC):
        lo = c * NL
        perm = perms[c]
        idx1 = np.empty(L1TOT, np.uint16)
        off = 0
        for t in range(NT):
            K = Ks[t]
            tbl = np.full((P, K), N, np.uint16)
            for p in range(P):
                l = t * P + p
                if l >= NL:
                    continue
                n0 = lo + int(perm[l])
                e0, e1 = rowptr[n0], rowptr[n0 + 1]
                tbl[p, :e1 - e0] = src_s[e0:e1]
            idx1[off:off + P * K] = tbl.ravel()
            off += P * K
        idx1s.append(idx1)

        psc = np.zeros((P, NT), np.int32)
        pix = np.zeros((P, NT), np.int32)
        btf = np.full((P, NT), -1.0, np.float32)
        for t in range(NT):
            for p in range(P):
                l = t * P + p
                if l < NL:
                    psc[p, t] = int(perm[l])
                    pix[p, t] = lo + int(perm[l])
                    btf[p, t] = float(batch[lo + perm[l]])
                else:
                    psc[p, t] = NL + p   # distinct trash rows
                    pix[p, t] = lo       # any valid node
        permscs.append(psc)
        pidxs.append(pix)
        btfs.append(btf)

    xshs = [x[c * NL:(c + 1) * NL].astype(ml_dtypes.bfloat16) for c in range(NC)]
    cnt = np.bincount(batch, minlength=G).astype(np.float32).reshape(G, 1)

    return dict(Ks=Ks, L1TOT=L1TOT, idx1s=idx1s, permscs=permscs,
                pidxs=pidxs, btfs=btfs, xshs=xshs, cnt=cnt)


def fold_weights(W1, att_src1, att_dst1, b1, W2, att_src2, att_dst2, b2,
                 Wh1, bh1, Wh2, bh2):
    import ml_dtypes
    W1 = np.asarray(W1, np.float32)
    W1r = W1.reshape(IN, H1, C1)
    Vs = np.einsum("fhc,hc->fh", W1r, np.asarray(att_src1, np.float32))
    Vd = np.einsum("fhc,hc->fh", W1r, np.asarray(att_dst1, np.float32))
    # A8 row layout: rows p=16g+f; cols 0:32 = a_s (g*4+h), cols 32:64 = a_d.
    A8_lhsT = np.zeros((P, 64), np.float32)
    for g in range(NC):
        A8_lhsT[g * IN:(g + 1) * IN, g * 4:(g + 1) * 4] = Vs
        A8_lhsT[g * IN:(g + 1) * IN, 32 + g * 4:32 + (g + 1) * 4] = Vd
    W1blk = np.zeros((64, F1), np.float32)
    for h in range(H1):
        W1blk[h * IN:(h + 1) * IN, h * C1:(h + 1) * C1] = W1r[:, h, :]
    att2 = np.stack([np.asarray(att_src2, np.float32).ravel(),
                     np.asarray(att_dst2, np.float32).ravel()], 1)  # [32, 2]
    b2bc = np.broadcast_to(np.asarray(b2, np.float32).reshape(1, C2),
                           (P, C2)).copy()
    return dict(A8w=A8_lhsT.astype(ml_dtypes.bfloat16), W1blk=W1blk,
                b1=np.asarray(b1, np.float32).reshape(F1, 1),
                W2=np.asarray(W2, np.float32), att2=att2, b2bc=b2bc,
                Wh1=np.asarray(Wh1, np.float32),
                bh1=np.asarray(bh1, np.float32).reshape(64, 1),
                Wh2=np.asarray(Wh2, np.float32),
                bh2=np.asarray(bh2, np.float32).reshape(1, 1))


# ---------------------------------------------------------------------------
def edge_softmax_aggregate(nc, pools, idx_dram, tbl_dram, a_d_view, K,
                           rec, nmsg, nheads, out_cb):
    """Per-tile padded-CSR gather + segment softmax + weighted aggregation.

    Table rows are bf16; compute is f32 after one conversion copy.
    a_d_view: AP [128, nheads] f32 (per-dst attention term, this tile)
    rec: record width; nmsg: message feature count (cols 0:nmsg of record);
    a_s lives at record col nmsg..nmsg+nheads-1.
    out_cb(OPS): callback receiving [128, nheads*nmsg] aggregated+normalized.
    """
    work = pools["work"]
    H = nheads
    itu = work.tile([P, K], U16, tag="itu")
    nc.sync.dma_start(out=itu[:], in_=idx_dram)
    it = work.tile([P, K], I32, tag="it")
    nc.vector.tensor_copy(out=it[:], in_=itu[:])
    g_ = work.tile([P, K * rec], BF16, tag="g")
    # HW indirect DMA consumes ONE offset per partition (per contiguous dest
    # run), so gather one k-slot (128 rows) per instruction.
    for k in range(K):
        nc.gpsimd.indirect_dma_start(
            out=g_[:, k * rec:(k + 1) * rec], out_offset=None, in_=tbl_dram,
            in_offset=bass.IndirectOffsetOnAxis(ap=it[:, k:k + 1], axis=0))
    gf = work.tile([P, K * rec], F32, tag="gf")
    nc.vector.tensor_copy(out=gf[:], in_=g_[:])

    # logits L0[p, h, k] = a_s[src] + a_d[dst]
    L0 = work.tile([P, H * K], F32, tag="L0")
    nc.vector.tensor_tensor(
        out=L0[:],
        in0=svap(gf, nmsg, [[1, H], [rec, K]]),
        in1=svap(a_d_view, 0, [[1, H], [0, K]]),
        op=OP.add)
    # leaky relu
    Lm = work.tile([P, H * K], F32, tag="Lm")
    nc.vector.tensor_scalar_mul(Lm[:], L0[:], NEG)
    nc.vector.tensor_tensor(out=Lm[:], in0=L0[:], in1=Lm[:], op=OP.max)
    # segment max / exp / denom
    m = work.tile([P, H], F32, tag="m")
    nc.vector.tensor_reduce(
        out=m[:], in_=svap(Lm, 0, [[K, H], [1, K]]),
        axis=AX.X, op=OP.max)
    S = work.tile([P, H * K], F32, tag="S")
    nc.vector.tensor_tensor(
        out=S[:], in0=Lm[:],
        in1=svap(m, 0, [[1, H], [0, K]]), op=OP.subtract)
    # clamp: pad slots carry ~-2e29 logits; HW ACT Exp tables need sane range
    nc.vector.tensor_scalar_max(S[:], S[:], -80.0)
    EX = work.tile([P, H * K], F32, tag="EX")
    nc.scalar.activation(EX[:], S[:], ACT.Exp)
    den = work.tile([P, H], F32, tag="den")
    nc.vector.tensor_reduce(
        out=den[:], in_=svap(EX, 0, [[K, H], [1, K]]),
        axis=AX.X, op=OP.add)
    dr = work.tile([P, H], F32, tag="dr")
    nc.vector.tensor_scalar_add(dr[:], den[:], 1e-16)
    nc.vector.reciprocal(dr[:], dr[:])
    # weighted aggregation: OP[p,h,f] = sum_k EX[p,h,k] * msg[p,k,f]
    prod = work.tile([P, H * K * nmsg], F32, tag="prod")
    nc.vector.tensor_tensor(
        out=prod[:],
        in0=svap(EX, 0, [[K, H], [1, K], [0, nmsg]]),
        in1=svap(gf, 0, [[0, H], [rec, K], [1, nmsg]]),
        op=OP.mult)
    agg = work.tile([P, H * nmsg], F32, tag="agg")
    nc.vector.tensor_reduce(
        out=agg[:],
        in_=svap(prod, 0, [[K * nmsg, H], [1, nmsg], [nmsg, K]]),
        axis=AX.X, op=OP.add)
    ops = work.tile([P, H * nmsg], F32, tag="ops")
    nc.vector.tensor_tensor(
        out=ops[:], in0=agg[:],
        in1=svap(dr, 0, [[1, H], [0, nmsg]]), op=OP.mult)
    out_cb(ops)


def elu_from(nc, work, zb, out_ap, tagp):
    """out = ELU(zb) = min(exp(min(zb,0))-1,0)+max(zb,0), written to out_ap."""
    tmin = work.tile(list(zb.shape), F32, tag=tagp + "min")
    nc.vector.tensor_scalar_min(tmin[:], zb[:], 0.0)
    te = work.tile(list(zb.shape), F32, tag=tagp + "e")
    nc.scalar.activation(te[:], tmin[:], ACT.Exp)
    trelu = work.tile(list(zb.shape), F32, tag=tagp + "r")
    nc.vector.tensor_scalar_max(trelu[:], zb[:], 0.0)
    nc.vector.scalar_tensor_tensor(
        out=out_ap, in0=te[:], scalar=-1.0, in1=trelu[:],
        op0=OP.add, op1=OP.add)


def build_kernel(Ks):
    nc = bass.Bass(num_devices=NC)
    L1TOT = P * sum(Ks)
    rg = [list(range(NC))]

    xsh_d = nc.declare_dram_parameter("xsh", [NL, IN], BF16, isOutput=False)
    idx1_d = nc.declare_dram_parameter("idx1", [L1TOT], U16, isOutput=False)
    psc_d = nc.declare_dram_parameter("permsc", [P, NT], I32, isOutput=False)
    pidx_d = nc.declare_dram_parameter("pidx", [P, NT], I32, isOutput=False)
    btf_d = nc.declare_dram_parameter("btf", [P, NT], F32, isOutput=False)
    A8w_d = nc.declare_dram_parameter("A8w", [P, 64], BF16, isOutput=False)
    W1blk_d = nc.declare_dram_parameter("W1blk", [64, F1], F32, isOutput=False)
    b1_d = nc.declare_dram_parameter("b1", [F1, 1], F32, isOutput=False)
    W2_d = nc.declare_dram_parameter("W2", [F1, C2], F32, isOutput=False)
    att2_d = nc.declare_dram_parameter("att2", [C2, 2], F32, isOutput=False)
    b2bc_d = nc.declare_dram_parameter("b2bc", [P, C2], F32, isOutput=False)
    Wh1_d = nc.declare_dram_parameter("Wh1", [C2, 64], F32, isOutput=False)
    bh1_d = nc.declare_dram_parameter("bh1", [64, 1], F32, isOutput=False)
    Wh2_d = nc.declare_dram_parameter("Wh2", [64, 1], F32, isOutput=False)
    bh2_d = nc.declare_dram_parameter("bh2", [1, 1], F32, isOutput=False)
    cnt_d = nc.declare_dram_parameter("cnt", [G, 1], F32, isOutput=False)
    out_d = nc.declare_dram_parameter("out", [1, G], F32, isOutput=True)

    xshb = nc.dram_tensor("xshb", [NL, IN], BF16)
    xfull = nc.dram_tensor("xfull", [N, IN], BF16)
    T1 = nc.dram_tensor("T1", [N + 1, REC1], BF16)
    astab = nc.dram_tensor("astab", [N, 4], BF16)
    adtab = nc.dram_tensor("adtab", [N, 4], BF16)
    t2loc = nc.dram_tensor("t2loc", [NL + P, REC2], BF16)
    T2full = nc.dram_tensor("T2full", [N + 1, REC2], BF16)
    adbuf = nc.dram_tensor("adbuf", [NLP], F32)
    pr_in = nc.dram_tensor("pr_in", [G, C2], F32)
    pr_out = nc.dram_tensor("pr_out", [G, C2], F32)

    with ctile.TileContext(nc) as tc:
        import contextlib
        with contextlib.ExitStack() as ctx:
            const = ctx.enter_context(tc.tile_pool(name="const", bufs=1))
            persist = ctx.enter_context(tc.tile_pool(name="persist", bufs=1))
            work = ctx.enter_context(tc.tile_pool(name="work", bufs=2))
            psum = ctx.enter_context(tc.tile_pool(name="psum", bufs=4, space="PSUM"))
            ppool = ctx.enter_context(tc.tile_pool(name="ppool", bufs=1, space="PSUM"))
            pools = dict(work=work, psum=psum)

            ident = const.tile([P, P], F32)
            make_identity(nc, ident[:])
            w1blk_s = const.tile([64, F1], F32)
            nc.sync.dma_start(out=w1blk_s[:], in_=W1blk_d[:])
            b1_s = const.tile([F1, 1], F32)
            nc.sync.dma_start(out=b1_s[:], in_=b1_d[:])
            w2_s = const.tile([F1, C2], F32)
            nc.sync.dma_start(out=w2_s[:], in_=W2_d[:])
            att2_s = const.tile([C2, 2], F32)
            nc.sync.dma_start(out=att2_s[:], in_=att2_d[:])
            b2bc_s = const.tile([P, C2], F32)
            nc.sync.dma_start(out=b2bc_s[:], in_=b2bc_d[:])
            btf_s = const.tile([P, NT], F32)
            nc.sync.dma_start(out=btf_s[:], in_=btf_d[:])
            psc_s = const.tile([P, NT], I32)
            nc.sync.dma_start(out=psc_s[:], in_=psc_d[:])
            pidx_s = const.tile([P, NT], I32)
            nc.sync.dma_start(out=pidx_s[:], in_=pidx_d[:])
            # iota row 0..G-1 on every partition, as f32
            io32 = const.tile([P, G], I32)
            nc.gpsimd.iota(io32[:], pattern=[[1, G]], base=0, channel_multiplier=0)
            iof = const.tile([P, G], F32)
            nc.vector.tensor_copy(out=iof[:], in_=io32[:])

            # ---- phase 0: x AllGather + T1 build ----
            nc.gpsimd.dma_start(out=xshb[:], in_=xsh_d[:])
            nc.gpsimd.collective_compute(
                "AllGather", OP.bypass, replica_groups=rg,
                ins=[xshb[:]], outs=[xfull[:]])
            # T1 x-columns (stride-REC1 rows from packed bf16 rows)
            nc.sync.dma_start(
                out=vap(T1, 0, [[REC1, N], [1, IN]]),
                in_=vap(xfull, 0, [[IN, N], [1, IN]]))
            # T1 dummy row: x=0, a_s=-1e30
            drow = const.tile([1, REC1], BF16)
            nc.vector.memset(drow[:, 0:IN], 0.0)
            nc.vector.memset(drow[:, IN:REC1], -1e30)
            nc.sync.dma_start(out=T1[N:N + 1, :], in_=drow[:])

            # xt[p=16g+f, j] = x[8j+g, f]; contiguous 128-elem runs
            xt = persist.tile([P, NL], BF16)
            nc.sync.dma_start(out=xt[:], in_=vap(xfull, 0, [[1, P], [P, NL]]))
            a8w_s = const.tile([P, 64], BF16)
            nc.sync.dma_start(out=a8w_s[:], in_=A8w_d[:])
            a8s = persist.tile([64, NL], BF16)
            for c0 in range(0, NL, CH):
                w = min(CH, NL - c0)
                pz = psum.tile([64, CH], F32, tag="ps")
                nc.tensor.matmul(pz[:, :w], lhsT=a8w_s[:], rhs=xt[:, c0:c0 + w],
                                 start=True, stop=True)
                nc.vector.tensor_copy(out=a8s[:, c0:c0 + w], in_=pz[:, :w])
            # a8s partition q=4g+v, col j <-> node 8j+g: astab offset 32j+q
            nc.sync.dma_start(
                out=vap(astab, 0, [[1, 32], [32, NL]]), in_=a8s[0:32, :])
            nc.sync.dma_start(
                out=vap(adtab, 0, [[1, 32], [32, NL]]), in_=a8s[32:64, :])
            nc.sync.dma_start(
                out=vap(T1, IN, [[REC1, N], [1, 4]]),
                in_=vap(astab, 0, [[4, N], [1, 4]]))

            # per-dst a_d in degree-sorted order: [128, NT*4]
            adS = persist.tile([P, NT * 4], BF16)
            for t in range(NT):
                nc.gpsimd.indirect_dma_start(
                    out=adS[:, t * 4:(t + 1) * 4], out_offset=None,
                    in_=adtab[:],
                    in_offset=bass.IndirectOffsetOnAxis(
                        ap=pidx_s[:, t:t + 1], axis=0))
            adSf = persist.tile([P, NT * 4], F32)
            nc.vector.tensor_copy(out=adSf[:], in_=adS[:])

            # ---- layer-1 edge phase ----
            h1e = persist.tile([F1, NLP], F32)
            off = 0
            for t in range(NT):
                K = Ks[t]
                idx_dram = vap(idx1_d, off, [[K, P], [1, K]])
                off += P * K

                def finish1(ops, t=t):
                    pt = psum.tile([64, P], F32, tag="ps")
                    nc.tensor.transpose(out=pt[:], in_=ops[:], identity=ident[:, :P])
                    opst = work.tile([64, P], F32, tag="opst")
                    nc.vector.tensor_copy(out=opst[:], in_=pt[:])
                    hz = psum.tile([F1, P], F32, tag="ps")
                    nc.tensor.matmul(hz[:], lhsT=w1blk_s[:], rhs=opst[:],
                                     start=True, stop=True)
                    zb = work.tile([F1, P], F32, tag="zb")
                    nc.scalar.activation(zb[:], hz[:], ACT.Identity, bias=b1_s[:])
                    elu_from(nc, work, zb, h1e[:, t * P:(t + 1) * P], "f1")

                edge_softmax_aggregate(
                    nc, pools, idx_dram, T1[:],
                    adSf[:, t * 4:(t + 1) * 4], K, REC1, IN, H1, finish1)

            # ---- layer-2 node phase ----
            h2a = persist.tile([C2 + 1, NLP], F32)
            adrow = persist.tile([1, NLP], F32)
            for c0 in range(0, NLP, CH):
                w = min(CH, NLP - c0)
                pz = psum.tile([C2, CH], F32, tag="ps")
                nc.tensor.matmul(pz[:, :w], lhsT=w2_s[:], rhs=h1e[:, c0:c0 + w],
                                 start=True, stop=True)
                nc.vector.tensor_copy(out=h2a[0:C2, c0:c0 + w], in_=pz[:, :w])
                pa = psum.tile([1, CH], F32, tag="ps")
                nc.tensor.matmul(pa[:, :w], lhsT=att2_s[:, 0:1],
                                 rhs=h2a[0:C2, c0:c0 + w], start=True, stop=True)
                nc.vector.tensor_copy(out=h2a[C2:C2 + 1, c0:c0 + w], in_=pa[:, :w])
                pb = psum.tile([1, CH], F32, tag="ps")
                nc.tensor.matmul(pb[:, :w], lhsT=att2_s[:, 1:2],
                                 rhs=h2a[0:C2, c0:c0 + w], start=True, stop=True)
                nc.vector.tensor_copy(out=adrow[:, c0:c0 + w], in_=pb[:, :w])

            # a_d2 reshaped to [128, NT] via a DRAM bounce
            nc.sync.dma_start(out=adbuf[:], in_=adrow[:])
            ad2t = persist.tile([P, NT], F32)
            nc.sync.dma_start(out=ad2t[:], in_=vap(adbuf, 0, [[1, P], [P, NT]]))

            # ---- T2 record assembly: scatter to original-node-order rows ----
            for t in range(NT):
                pt = psum.tile([P, C2 + 1], F32, tag="ps")
                nc.tensor.transpose(
                    out=pt[:], in_=h2a[:, t * P:(t + 1) * P],
                    identity=ident[0:C2 + 1, 0:C2 + 1])
                rec = work.tile([P, REC2], BF16, tag="rec")
                nc.vector.tensor_copy(out=rec[:, 0:C2 + 1], in_=pt[:])
                nc.vector.memset(rec[:, C2 + 1:REC2], 0.0)
                nc.gpsimd.indirect_dma_start(
                    out=t2loc[:], out_offset=bass.IndirectOffsetOnAxis(
                        ap=psc_s[:, t:t + 1], axis=0),
                    in_=rec[:], in_offset=None)

            # AllGather local [NL, REC2] slices -> full table rows 0..N-1
            nc.gpsimd.collective_compute(
                "AllGather", OP.bypass, replica_groups=rg,
                ins=[vap(t2loc, 0, [[1, NL * REC2]])],
                outs=[vap(T2full, 0, [[1, N * REC2]])])
            drow2 = const.tile([1, REC2], BF16)
            nc.vector.memset(drow2[:, 0:C2], 0.0)
            nc.vector.memset(drow2[:, C2:C2 + 1], -1e30)
            nc.vector.memset(drow2[:, C2 + 1:REC2], 0.0)
            nc.sync.dma_start(out=T2full[N:N + 1, :], in_=drow2[:])

            # ---- layer-2 edge phase (same indices as layer 1) ----
            pooled = ppool.tile([G, C2], F32)
            off = 0
            for t in range(NT):
                K = Ks[t]
                idx_dram = vap(idx1_d, off, [[K, P], [1, K]])
                off += P * K

                def finish2(ops, t=t):
                    zb = work.tile([P, C2], F32, tag="zb2")
                    nc.vector.tensor_tensor(out=zb[:], in0=ops[:], in1=b2bc_s[:],
                                            op=OP.add)
                    hf = work.tile([P, C2], F32, tag="hf")
                    elu_from(nc, work, zb, hf[:], "f2")
                    oh = work.tile([P, G], F32, tag="oh")
                    nc.vector.tensor_tensor(
                        out=oh[:],
                        in0=svap(btf_s, t, [[0, G]]),
                        in1=iof[:], op=OP.is_equal)
                    nc.tensor.matmul(
                        pooled[:], lhsT=oh[:], rhs=hf[:],
                        start=(t == 0), stop=(t == NT - 1))

                edge_softmax_aggregate(
                    nc, pools, idx_dram, T2full[:],
                    ad2t[:, t:t + 1], K, REC2, C2, 1, finish2)

            # ---- AllReduce partial pooled sums; replicated MLP head ----
            po = const.tile([G, C2], F32)
            nc.vector.tensor_copy(out=po[:], in_=pooled[:])
            nc.sync.dma_start(out=pr_in[:], in_=po[:])
            nc.gpsimd.collective_compute(
                "AllReduce", OP.add, replica_groups=rg,
                ins=[pr_in[:]], outs=[pr_out[:]])
            sums = const.tile([G, C2], F32)
            nc.sync.dma_start(out=sums[:], in_=pr_out[:])

            cntt = const.tile([G, 1], F32)
            nc.sync.dma_start(out=cntt[:], in_=cnt_d[:])
            wh1 = const.tile([C2, 64], F32)
            nc.sync.dma_start(out=wh1[:], in_=Wh1_d[:])
            bh1 = const.tile([64, 1], F32)
            nc.sync.dma_start(out=bh1[:], in_=bh1_d[:])
            wh2 = const.tile([64, 1], F32)
            nc.sync.dma_start(out=wh2[:], in_=Wh2_d[:])
            bh2 = const.tile([1, 1], F32)
            nc.sync.dma_start(out=bh2[:], in_=bh2_d[:])

            cm = const.tile([G, 1], F32)
            nc.vector.tensor_scalar_max(cm[:], cntt[:], 1.0)
            nc.vector.reciprocal(cm[:], cm[:])
            pooledm = const.tile([G, C2], F32)
            nc.vector.tensor_scalar_mul(pooledm[:], sums[:], cm[:])

            pt = ppool.tile([C2, G], F32, tag="fin")
            nc.tensor.transpose(out=pt[:], in_=pooledm[:], identity=ident[:G, :G])
            pooledT = const.tile([C2, G], F32)
            nc.vector.tensor_copy(out=pooledT[:], in_=pt[:])
            z1 = ppool.tile([64, G], F32, tag="fin2")
            nc.tensor.matmul(z1[:], lhsT=wh1[:], rhs=pooledT[:], start=True,
                             stop=True)
            r1 = const.tile([64, G], F32)
            nc.scalar.activation(r1[:], z1[:], ACT.Relu, bias=bh1[:])
            z2 = ppool.tile([1, G], F32, tag="fin3")
            nc.tensor.matmul(z2[:], lhsT=wh2[:], rhs=r1[:], start=True, stop=True)
            o = const.tile([1, G], F32)
            nc.scalar.activation(o[:], z2[:], ACT.Identity, bias=bh2[:])
            nc.sync.dma_start(out=out_d[:], in_=o[:])
    fix_multiwait(nc)
    return nc


# ---------------------------------------------------------------------------
def make_inmaps(prep, fw):
    in1 = []
    for c in range(NC):
        in1.append(dict(
            xsh=prep["xshs"][c], idx1=prep["idx1s"][c],
            permsc=prep["permscs"][c], pidx=prep["pidxs"][c],
            btf=prep["btfs"][c], cnt=prep["cnt"], **fw))
    return in1


def kernel(x, edge_index, batch, W1, att_src1, att_dst1, b1,
           W2, att_src2, att_dst2, b2, Wh1, bh1, Wh2, bh2):
    prep = host_prep(x, edge_index, batch)
    fw = fold_weights(W1, att_src1, att_dst1, b1, W2, att_src2, att_dst2, b2,
                      Wh1, bh1, Wh2, bh2)
    nc = build_kernel(prep["Ks"])
    res = run_bass_kernel_spmd(nc, make_inmaps(prep, fw), list(range(NC)))
    return res.results[0]["out"].reshape(G, 1).astype(np.float32)


def _wall_min(fn, n=5):
    import time
    best = 1e9
    for _ in range(n):
        t0 = time.perf_counter()
        fn()
        best = min(best, time.perf_counter() - t0)
    return best


def _null_nc():
    nc = bass.Bass()
    x = nc.declare_dram_parameter("x", [P, 64], F32, isOutput=False)
    y = nc.declare_dram_parameter("y", [P, 64], F32, isOutput=True)
    with ctile.TileContext(nc) as tc:
        with tc.tile_pool(name="sbuf", bufs=1) as pool:
            t = pool.tile([P, 64], F32)
            nc.sync.dma_start(out=t[:], in_=x[:])
            nc.sync.dma_start(out=y[:], in_=t[:])
    fix_multiwait(nc)
    return nc


def timed_run(inputs):
    """Estimate on-device exec ns: warm per-call wall minus null-kernel wall.

    The axon PJRT path exposes no NTFF profiling and the For_i loop repeat
    trick does not compile on this toolchain, so this is an upper-bound
    estimate: warm wall of the single launch minus the warm wall of a trivial
    kernel (same dispatch/tunnel overhead), floored at 0.
    """
    prep = host_prep(inputs["x"], inputs["edge_index"], inputs["batch"])
    fw = fold_weights(inputs["W1"], inputs["att_src1"], inputs["att_dst1"],
                      inputs["b1"], inputs["W2"], inputs["att_src2"],
                      inputs["att_dst2"], inputs["b2"], inputs["Wh1"],
                      inputs["bh1"], inputs["Wh2"], inputs["bh2"])
    cores = list(range(NC))
    in1 = make_inmaps(prep, fw)

    nc0 = _null_nc()
    im0 = [dict(x=np.zeros((P, 64), np.float32)) for _ in range(NC)]
    run_bass_kernel_spmd(nc0, im0, cores)
    t0 = _wall_min(lambda: run_bass_kernel_spmd(nc0, im0, cores), n=5)

    nc1 = build_kernel(prep["Ks"])
    run_bass_kernel_spmd(nc1, in1, cores)
    t1 = _wall_min(lambda: run_bass_kernel_spmd(nc1, in1, cores), n=5)

    d1 = max(t1 - t0, 0.0)
    print(f"null wall {t0*1e3:.1f} ms; launch wall {t1*1e3:.1f} ms")
    print(f"launch exec est {d1*1e6:.0f} us")
    return d1 * 1e9


# revision 15
# speedup vs baseline: 11.2099x; 1.2721x over previous
"""GAT regressor (2x GATConv + mean-pool + MLP) on 8 Trainium2 cores.

Single-launch, dst-sharded, aggregate-then-transform, instruction-count-lean:
- Edges sorted by destination with SELF-LOOPS FIRST in each dst's run, so the
  k=0 CSR slot of every dst row is its self-loop: the gathered k=0 record
  carries the destination's own a_d term (no separate a_d gather).
- Core c owns dst nodes [c*6250, (c+1)*6250), renumbered by descending
  in-degree; padded CSR = one [128 x K_t] slot grid per 128 nodes.
- x is sharded across cores (bf16) and AllGathered on device.
- Layer 1 aggregates the 16-dim inputs (W1 applied after); per-edge gather is
  a 48B bf16 record [x(16), a_s1(4), a_d1(4)] via one indirect DMA per slot
  column. Padding slots point at a dummy row with a_s=-400, which kills them
  through leaky-relu+exp without any clamp or max-subtraction (logits are
  O(1), so softmax without the max shift is exact).
- Per-node logit terms a_s/a_d are folded matvecs (x @ (W1 @ att)) computed
  on-device with one group-packed K=128 bf16 matmul.
- att2 is folded into the W2 transform (w2aug = [W2, W2@att_s2, W2@att_d2]),
  so layer-2 records [h2(32), a_s2(1), a_d2(1), pad(2)] come out of a single
  matmul; records are indirect-scattered back to original node order and
  AllGathered, so layer 2 reuses layer 1's gather indices verbatim.
- Graph mean-pool via a single iota/is_equal one-hot matmul chain, partial
  [G, C2] AllReduced across cores; tiny MLP head replicated.
- ELU and bias chains run once over whole-layer staging buffers, not per tile.
- Host work: index/layout preprocessing + weight folding only.
"""
import numpy as np

import concourse.bass as bass
import concourse.tile as ctile
from concourse import mybir
from concourse.vector_clock import ScopedClock
from concourse.bass_utils import run_bass_kernel_spmd
from concourse.masks import make_identity

F32 = mybir.dt.float32
BF16 = mybir.dt.bfloat16
I32 = mybir.dt.int32
U16 = mybir.dt.uint16
AX = mybir.AxisListType
OP = mybir.AluOpType
ACT = mybir.ActivationFunctionType

N = 50000
E0 = 1_600_000
G = 100
IN = 16
H1, C1 = 4, 32
F1 = H1 * C1              # 128
C2 = 32
NEG = 0.2
NC = 8
NL = N // NC              # 6250
P = 128
NT = (NL + P - 1) // P    # 49
NLP = NT * P              # 6272
REC1 = 24                 # [x(16), a_s1(4), a_d1(4)]
REC2 = 36                 # [h2(32), a_s2(1), a_d2(1), pad(2)]
CH = 512
APAD = -400.0             # pad-slot a_s: leaky*APAD -> exp ~ 1.8e-35


# ---------------------------------------------------------------------------
# TileContext tail-drain patch: this walrus build allows only one sem wait per
# CTRL instruction; spread the kernel-tail drain waits over several drains.
def _patched_drain_and_barrier(self, tick_clock, wait_clock):
    drain_inst = self.nc.sync.drain()
    extras = [self.nc.sync.drain() for _ in range(40)]
    wait_clock.add_sem_waits(
        drain_inst.ins, ScopedClock({None: tick_clock.global_clock})
    )
    si = drain_inst.ins.sync_info
    waits = list(si.on_wait or []) if si is not None else []
    if len(waits) > 1:
        si.on_wait = waits[:1]
        for i, w in enumerate(waits[1:]):
            esi = extras[i].ins.sync_info
            if esi is None:
                extras[i].ins.sync_info = mybir.SyncInfo(on_wait=[w], on_update=[])
            else:
                esi.on_wait = [w]
    self.nc.all_engine_barrier()
    popped = self.nc._tile_sem_poison_stack.pop()
    assert popped is self._sem_poison
    self.nc.clear_and_free_semaphores(list(self.sems.allocated().values()))
    self.nc.all_engine_barrier()


ctile.TileContext._drain_and_barrier = _patched_drain_and_barrier


def fix_multiwait(nc):
    """This walrus build allows only one sem wait per instruction: hoist all
    but one wait of any instruction onto same-engine NOPs inserted before it."""
    for f in nc.m.functions:
        for bb in f.blocks:
            lst = bb.instructions
            i = 0
            while i < len(lst):
                inst = lst[i]
                si = inst.sync_info
                waits = list(si.on_wait) if si and si.on_wait else []
                if len(waits) > 1:
                    si.on_wait = waits[-1:]
                    for w in waits[:-1]:
                        nop = mybir.InstNoOp(
                            name=nc.get_next_instruction_name(), ins=[], outs=[])
                        nop.engine = inst.engine
                        nop.sync_info = mybir.SyncInfo(on_wait=[w], on_update=[])
                        nc.register_instruction(nop)
                        lst.insert(i, nop)
                        i += 1
                i += 1


def vap(t, off, dims):
    """Flat (DRAM) AP view with extra element offset and [step,count] dims."""
    a = t[:] if not isinstance(t, bass.AP) else t
    return bass.AP(tensor=a.tensor, offset=a.offset + off, ap=dims)


def svap(t, off, free_dims):
    """SBUF AP view: keeps the base AP's partition pair (partition step must
    stay the tile's free pitch), custom free [step,count] dims + elem offset."""
    a = t[:] if not isinstance(t, bass.AP) else t
    return bass.AP(tensor=a.tensor, offset=a.offset + off,
                   ap=[list(a.ap[0])] + free_dims)


# ---------------------------------------------------------------------------
# host preprocessing: pure index/layout work
def host_prep(x, edge_index, batch):
    import ml_dtypes
    x = np.asarray(x, np.float32)
    ei = np.asarray(edge_index).astype(np.int64)
    batch = np.asarray(batch).astype(np.int64)

    # self-loops FIRST so each dst's run begins with its self-loop (k=0 slot)
    src = np.concatenate([np.arange(N, dtype=np.int64), ei[0]]).astype(np.int32)
    dst = np.concatenate([np.arange(N, dtype=np.int64), ei[1]]).astype(np.int32)
    order = np.argsort(dst, kind="stable")
    src_s, dst_s = src[order], dst[order]
    deg = np.bincount(dst_s, minlength=N)
    rowptr = np.zeros(N + 1, np.int64)
    np.cumsum(deg, out=rowptr[1:])

    perms = []
    deg_sorted_all = []
    for c in range(NC):
        lo = c * NL
        d_local = deg[lo:lo + NL]
        perm = np.argsort(-d_local, kind="stable").astype(np.int32)
        perms.append(perm)
        deg_sorted_all.append(d_local[perm])

    # global per-tile K schedule (shared program across cores)
    Ks = []
    for t in range(NT):
        k = 0
        for c in range(NC):
            seg = deg_sorted_all[c][t * P:(t + 1) * P]
            if len(seg):
                k = max(k, int(seg.max()))
        Ks.append(max(4, ((k + 3) // 4) * 4))
    L1TOT = P * sum(Ks)

    idx1s, permscs, btfs = [], [], []
    for c in range(NC):
        lo = c * NL
        perm = perms[c]
        cols = []
        for t in range(NT):
            K = Ks[t]
            tbl = np.full((P, K), N, np.uint16)
            for p in range(P):
                l = t * P + p
                if l >= NL:
                    continue
                n0 = lo + int(perm[l])
                e0, e1 = rowptr[n0], rowptr[n0 + 1]
                tbl[p, :e1 - e0] = src_s[e0:e1]
            cols.append(tbl)
        # [P, sum(Ks)] row-major: tile t occupies columns off_t..off_t+K_t
        idx1s.append(np.concatenate(cols, axis=1).ravel())

        psc = np.zeros((P, NT), np.int32)
        btf = np.full((P, NT), -1.0, np.float32)
        for t in range(NT):
            for p in range(P):
                l = t * P + p
                if l < NL:
                    psc[p, t] = int(perm[l])
                    btf[p, t] = float(batch[lo + perm[l]])
                else:
                    psc[p, t] = NL + p   # distinct trash rows
        permscs.append(psc)
        btfs.append(btf)

    xshs = [x[c * NL:(c + 1) * NL].astype(ml_dtypes.bfloat16) for c in range(NC)]
    cnt = np.bincount(batch, minlength=G).astype(np.float32).reshape(G, 1)
    iota = np.broadcast_to(np.arange(G, dtype=np.float32), (P, G)).copy()

    return dict(Ks=Ks, L1TOT=L1TOT, idx1s=idx1s, permscs=permscs,
                btfs=btfs, xshs=xshs, cnt=cnt, iota=iota)


def fold_weights(W1, att_src1, att_dst1, b1, W2, att_src2, att_dst2, b2,
                 Wh1, bh1, Wh2, bh2):
    import ml_dtypes
    W1 = np.asarray(W1, np.float32)
    W1r = W1.reshape(IN, H1, C1)
    Vs = np.einsum("fhc,hc->fh", W1r, np.asarray(att_src1, np.float32))
    Vd = np.einsum("fhc,hc->fh", W1r, np.asarray(att_dst1, np.float32))
    # A8 rows p=16g+f; col c=8g+h: h<4 -> Vs[:,h], h>=4 -> Vd[:,h-4], so the
    # [64, NL] result DMA-scatters to the interleaved [N, 8] a_s/a_d table
    # with one affine AP.
    A8_lhsT = np.zeros((P, 64), np.float32)
    for g in range(NC):
        A8_lhsT[g * IN:(g + 1) * IN, 8 * g:8 * g + 4] = Vs
        A8_lhsT[g * IN:(g + 1) * IN, 8 * g + 4:8 * g + 8] = Vd
    W1blk = np.zeros((64, F1), np.float32)
    for h in range(H1):
        W1blk[h * IN:(h + 1) * IN, h * C1:(h + 1) * C1] = W1r[:, h, :]
    W2 = np.asarray(W2, np.float32)
    w2aug = np.concatenate([
        W2,
        W2 @ np.asarray(att_src2, np.float32).reshape(C2, 1),
        W2 @ np.asarray(att_dst2, np.float32).reshape(C2, 1)], 1)  # [F1, 34]
    b2bc = np.broadcast_to(np.asarray(b2, np.float32).reshape(1, C2),
                           (P, C2)).copy()
    return dict(A8w=A8_lhsT.astype(ml_dtypes.bfloat16), W1blk=W1blk,
                b1=np.asarray(b1, np.float32).reshape(F1, 1),
                w2aug=w2aug, b2bc=b2bc,
                Wh1=np.asarray(Wh1, np.float32),
                bh1=np.asarray(bh1, np.float32).reshape(64, 1),
                Wh2=np.asarray(Wh2, np.float32),
                bh2=np.asarray(bh2, np.float32).reshape(1, 1))


NO_CC = False  # profiling aid: skip collectives (results invalid, timing valid)
GATHER_OFF = False   # profiling aid: replace indirect gathers with bulk DMA


def build_kernel(Ks):
    nc = bass.Bass(num_devices=NC)
    L1TOT = P * sum(Ks)
    SK = sum(Ks)
    rg = [list(range(NC))]

    def cc(kind, op, ins, outs):
        if NO_CC:
            nc.gpsimd.dma_start(
                out=bass.AP(tensor=outs[0].tensor, offset=outs[0].offset,
                            ap=ins[0].ap),
                in_=ins[0])
        else:
            nc.gpsimd.collective_compute(kind, op, replica_groups=rg,
                                         ins=ins, outs=outs)

    xsh_d = nc.declare_dram_parameter("xsh", [NL, IN], BF16, isOutput=False)
    idx1_d = nc.declare_dram_parameter("idx1", [L1TOT], U16, isOutput=False)
    psc_d = nc.declare_dram_parameter("permsc", [P, NT], I32, isOutput=False)
    btf_d = nc.declare_dram_parameter("btf", [P, NT], F32, isOutput=False)
    iota_d = nc.declare_dram_parameter("iota", [P, G], F32, isOutput=False)
    A8w_d = nc.declare_dram_parameter("A8w", [P, 64], BF16, isOutput=False)
    W1blk_d = nc.declare_dram_parameter("W1blk", [64, F1], F32, isOutput=False)
    b1_d = nc.declare_dram_parameter("b1", [F1, 1], F32, isOutput=False)
    w2aug_d = nc.declare_dram_parameter("w2aug", [F1, C2 + 2], F32,
                                        isOutput=False)
    b2bc_d = nc.declare_dram_parameter("b2bc", [P, C2], F32, isOutput=False)
    Wh1_d = nc.declare_dram_parameter("Wh1", [C2, 64], F32, isOutput=False)
    bh1_d = nc.declare_dram_parameter("bh1", [64, 1], F32, isOutput=False)
    Wh2_d = nc.declare_dram_parameter("Wh2", [64, 1], F32, isOutput=False)
    bh2_d = nc.declare_dram_parameter("bh2", [1, 1], F32, isOutput=False)
    cnt_d = nc.declare_dram_parameter("cnt", [G, 1], F32, isOutput=False)
    out_d = nc.declare_dram_parameter("out", [1, G], F32, isOutput=True)

    xshb = nc.dram_tensor("xshb", [NL, IN], BF16)
    xfull = nc.dram_tensor("xfull", [N, IN], BF16)
    T1 = nc.dram_tensor("T1", [N + 1, REC1], BF16)
    asdtab = nc.dram_tensor("asdtab", [N, 8], BF16)
    t2loc = nc.dram_tensor("t2loc", [NL + P, REC2], BF16)
    T2full = nc.dram_tensor("T2full", [N + 1, REC2], BF16)
    pr_in = nc.dram_tensor("pr_in", [G, C2], F32)
    pr_out = nc.dram_tensor("pr_out", [G, C2], F32)

    with ctile.TileContext(nc) as tc:
        import contextlib
        with contextlib.ExitStack() as ctx:
            const = ctx.enter_context(tc.tile_pool(name="const", bufs=1))
            persist = ctx.enter_context(tc.tile_pool(name="persist", bufs=1))
            work = ctx.enter_context(tc.tile_pool(name="work", bufs=2))
            psum = ctx.enter_context(tc.tile_pool(name="psum", bufs=4,
                                                  space="PSUM"))
            ppool = ctx.enter_context(tc.tile_pool(name="ppool", bufs=1,
                                                   space="PSUM"))

            ident = const.tile([P, P], F32)
            make_identity(nc, ident[:])
            w1blk_s = const.tile([64, F1], F32)
            nc.sync.dma_start(out=w1blk_s[:], in_=W1blk_d[:])
            b1_s = const.tile([F1, 1], F32)
            nc.sync.dma_start(out=b1_s[:], in_=b1_d[:])
            w2_s = const.tile([F1, C2 + 2], F32)
            nc.sync.dma_start(out=w2_s[:], in_=w2aug_d[:])
            b2bc_s = const.tile([P, C2], F32)
            nc.sync.dma_start(out=b2bc_s[:], in_=b2bc_d[:])
            btf_s = const.tile([P, NT], F32)
            nc.sync.dma_start(out=btf_s[:], in_=btf_d[:])
            psc_s = const.tile([P, NT], I32)
            nc.sync.dma_start(out=psc_s[:], in_=psc_d[:])
            iof = const.tile([P, G], F32)
            nc.sync.dma_start(out=iof[:], in_=iota_d[:])

            # whole-layer slot indices: [P, sum(Ks)] (tile t at col off_t)
            itu = const.tile([P, SK], U16)
            nc.sync.dma_start(out=itu[:], in_=vap(idx1_d, 0, [[SK, P], [1, SK]]))
            itA = const.tile([P, SK], I32)
            nc.vector.tensor_copy(out=itA[:], in_=itu[:])

            # ---- phase 0: x AllGather + T1 build + logit-term matmul ----
            nc.gpsimd.dma_start(out=xshb[:], in_=xsh_d[:])
            cc("AllGather", OP.bypass, [xshb[:]], [xfull[:]])
            nc.sync.dma_start(
                out=vap(T1, 0, [[REC1, N], [1, IN]]),
                in_=vap(xfull, 0, [[IN, N], [1, IN]]))
            drow = const.tile([1, REC1], BF16)
            nc.vector.memset(drow[:, 0:IN], 0.0)
            nc.vector.memset(drow[:, IN:IN + 4], APAD)
            nc.vector.memset(drow[:, IN + 4:REC1], 0.0)
            nc.sync.dma_start(out=T1[N:N + 1, :], in_=drow[:])

            with tc.tile_pool(name="phasea", bufs=1) as pha:
                # xt[p=16g+f, j] = x[8j+g, f]; contiguous 128-elem runs
                xt = pha.tile([P, NL], BF16)
                nc.sync.dma_start(out=xt[:],
                                  in_=vap(xfull, 0, [[1, P], [P, NL]]))
                a8w_s = pha.tile([P, 64], BF16)
                nc.sync.dma_start(out=a8w_s[:], in_=A8w_d[:])
                a8s = pha.tile([64, NL], BF16)
                for c0 in range(0, NL, CH):
                    w = min(CH, NL - c0)
                    pz = psum.tile([64, CH], F32, tag="ps")
                    nc.tensor.matmul(pz[:, :w], lhsT=a8w_s[:],
                                     rhs=xt[:, c0:c0 + w],
                                     start=True, stop=True)
                    nc.scalar.activation(a8s[:, c0:c0 + w], pz[:, :w],
                                         ACT.Identity)
                # a8s col c=8g+h, free j  ->  asdtab[8j+g, h] at 64j + c
                nc.sync.dma_start(
                    out=vap(asdtab, 0, [[1, 64], [64, NL]]), in_=a8s[:])
            nc.sync.dma_start(
                out=vap(T1, IN, [[REC1, N], [1, 8]]),
                in_=vap(asdtab, 0, [[8, N], [1, 8]]))

            # ---- layer-1 edge phase ----
            h1z = persist.tile([F1, NLP], F32)
            off = 0
            for t in range(NT):
                K = Ks[t]
                g_ = work.tile([P, K * REC1], BF16, tag="g")
                if GATHER_OFF:
                    nc.sync.dma_start(
                        out=g_[:],
                        in_=vap(T1, 0, [[REC1, P], [1, K * REC1]]))
                else:
                    for k in range(K):
                        nc.gpsimd.indirect_dma_start(
                            out=g_[:, k * REC1:(k + 1) * REC1],
                            out_offset=None, in_=T1[:],
                            in_offset=bass.IndirectOffsetOnAxis(
                                ap=itA[:, off + k:off + k + 1], axis=0))
                off += K
                gf = work.tile([P, K * REC1], F32, tag="gf")
                nc.vector.tensor_copy(out=gf[:], in_=g_[:])
                # logits (h-major, k-minor): a_s[src_k] + a_d[dst] (k=0 slot)
                L0 = work.tile([P, H1 * K], F32, tag="L0")
                nc.vector.tensor_tensor(
                    out=L0[:],
                    in0=svap(gf, IN, [[1, H1], [REC1, K]]),
                    in1=svap(gf, IN + 4, [[1, H1], [0, K]]),
                    op=OP.add)
                Lm = work.tile([P, H1 * K], F32, tag="Lm")
                nc.scalar.activation(Lm[:], L0[:], ACT.Lrelu, alpha=NEG)
                EX = work.tile([P, H1 * K], F32, tag="EX")
                nc.scalar.activation(EX[:], Lm[:], ACT.Exp)
                den = work.tile([P, H1], F32, tag="den")
                nc.vector.tensor_reduce(
                    out=den[:], in_=svap(EX, 0, [[K, H1], [1, K]]),
                    axis=AX.X, op=OP.add)
                dr = work.tile([P, H1], F32, tag="dr")
                nc.vector.reciprocal(dr[:], den[:])
                prod = work.tile([P, H1 * K * IN], F32, tag="prod")
                nc.vector.tensor_tensor(
                    out=prod[:],
                    in0=svap(EX, 0, [[K, H1], [1, K], [0, IN]]),
                    in1=svap(gf, 0, [[0, H1], [REC1, K], [1, IN]]),
                    op=OP.mult)
                agg = work.tile([P, H1 * IN], F32, tag="agg")
                nc.vector.tensor_reduce(
                    out=agg[:],
                    in_=svap(prod, 0, [[K * IN, H1], [1, IN], [IN, K]]),
                    axis=AX.X, op=OP.add)
                ops = work.tile([P, H1 * IN], F32, tag="ops")
                nc.vector.tensor_tensor(
                    out=ops[:], in0=agg[:],
                    in1=svap(dr, 0, [[1, H1], [0, IN]]), op=OP.mult)
                # transform: h1z[:, tile] = W1blk^T @ ops^T
                pt = psum.tile([64, P], F32, tag="ps")
                nc.tensor.transpose(out=pt[:], in_=ops[:], identity=ident[:, :P])
                opst = work.tile([64, P], F32, tag="opst")
                nc.vector.tensor_copy(out=opst[:], in_=pt[:])
                hz = psum.tile([F1, P], F32, tag="ps")
                nc.tensor.matmul(hz[:], lhsT=w1blk_s[:], rhs=opst[:],
                                 start=True, stop=True)
                nc.scalar.activation(h1z[:, t * P:(t + 1) * P], hz[:],
                                     ACT.Identity, bias=b1_s[:])

            # ELU over the whole layer at once (in place; h1e := h1z)
            etmp = persist.tile([F1, NLP], F32)
            nc.vector.tensor_scalar_min(etmp[:], h1z[:], 0.0)
            nc.scalar.activation(etmp[:], etmp[:], ACT.Exp)
            nc.vector.tensor_scalar_max(h1z[:], h1z[:], 0.0)
            nc.vector.scalar_tensor_tensor(
                out=h1z[:], in0=etmp[:], scalar=-1.0, in1=h1z[:],
                op0=OP.add, op1=OP.add)

            # ---- layer-2 node phase: [h2(32), a_s2, a_d2] in one matmul ----
            h2a = persist.tile([C2 + 2, NLP], F32)
            for c0 in range(0, NLP, CH):
                w = min(CH, NLP - c0)
                pz = psum.tile([C2 + 2, CH], F32, tag="ps")
                nc.tensor.matmul(pz[:, :w], lhsT=w2_s[:], rhs=h1z[:, c0:c0 + w],
                                 start=True, stop=True)
                nc.scalar.activation(h2a[:, c0:c0 + w], pz[:, :w], ACT.Identity)

            # ---- T2 record assembly: scatter to original-node-order rows ----
            recAll = persist.tile([P, NT * REC2], BF16)
            for t in range(NT):
                pt = psum.tile([P, C2 + 2], F32, tag="ps")
                nc.tensor.transpose(
                    out=pt[:], in_=h2a[:, t * P:(t + 1) * P],
                    identity=ident[0:C2 + 2, 0:C2 + 2])
                nc.scalar.activation(
                    recAll[:, t * REC2:t * REC2 + C2 + 2], pt[:], ACT.Identity)
            nc.vector.memset(
                svap(recAll, C2 + 2, [[REC2, NT], [1, REC2 - C2 - 2]]), 0.0)
            for t in range(NT):
                nc.gpsimd.indirect_dma_start(
                    out=t2loc[:], out_offset=bass.IndirectOffsetOnAxis(
                        ap=psc_s[:, t:t + 1], axis=0),
                    in_=recAll[:, t * REC2:(t + 1) * REC2], in_offset=None)

            cc("AllGather", OP.bypass,
               [vap(t2loc, 0, [[1, NL * REC2]])],
               [vap(T2full, 0, [[1, N * REC2]])])
            drow2 = const.tile([1, REC2], BF16)
            nc.vector.memset(drow2[:, 0:C2], 0.0)
            nc.vector.memset(drow2[:, C2:C2 + 1], APAD)
            nc.vector.memset(drow2[:, C2 + 1:REC2], 0.0)
            nc.sync.dma_start(out=T2full[N:N + 1, :], in_=drow2[:])

            # ---- layer-2 edge phase (same slot indices as layer 1) ----
            hfA = persist.tile([P, NT * C2], F32)
            off = 0
            for t in range(NT):
                K = Ks[t]
                g_ = work.tile([P, K * REC2], BF16, tag="g")
                if GATHER_OFF:
                    nc.sync.dma_start(
                        out=g_[:],
                        in_=vap(T2full, 0, [[REC2, P], [1, K * REC2]]))
                else:
                    for k in range(K):
                        nc.gpsimd.indirect_dma_start(
                            out=g_[:, k * REC2:(k + 1) * REC2],
                            out_offset=None, in_=T2full[:],
                            in_offset=bass.IndirectOffsetOnAxis(
                                ap=itA[:, off + k:off + k + 1], axis=0))
                off += K
                gf = work.tile([P, K * REC2], F32, tag="gf")
                nc.vector.tensor_copy(out=gf[:], in_=g_[:])
                # Lrelu(a_s[src_k] + a_d[dst]) with a_d as per-partition bias
                Lm = work.tile([P, K], F32, tag="Lm")
                nc.scalar.activation(
                    Lm[:], svap(gf, C2, [[REC2, K]]), ACT.Lrelu,
                    bias=svap(gf, C2 + 1, [[1, 1]]), alpha=NEG)
                EX = work.tile([P, K], F32, tag="EX")
                den = work.tile([P, 1], F32, tag="den")
                nc.scalar.activation(EX[:], Lm[:], ACT.Exp, accum_out=den[:])
                dr = work.tile([P, 1], F32, tag="dr")
                nc.vector.reciprocal(dr[:], den[:])
                prod = work.tile([P, K * C2], F32, tag="prod")
                nc.vector.tensor_tensor(
                    out=prod[:],
                    in0=svap(EX, 0, [[1, K], [0, C2]]),
                    in1=svap(gf, 0, [[REC2, K], [1, C2]]),
                    op=OP.mult)
                agg = work.tile([P, C2], F32, tag="agg")
                nc.vector.tensor_reduce(
                    out=agg[:], in_=svap(prod, 0, [[1, C2], [C2, K]]),
                    axis=AX.X, op=OP.add)
                nc.vector.tensor_tensor(
                    out=hfA[:, t * C2:(t + 1) * C2], in0=agg[:],
                    in1=svap(dr, 0, [[0, C2]]), op=OP.mult)

            # bias + ELU over the whole layer, then pool
            nc.vector.tensor_tensor(
                out=hfA[:], in0=hfA[:],
                in1=svap(b2bc_s, 0, [[0, NT], [1, C2]]), op=OP.add)
            etmp2 = etmp[:, 0:NT * C2]
            nc.vector.tensor_scalar_min(etmp2, hfA[:], 0.0)
            nc.scalar.activation(etmp2, etmp2, ACT.Exp)
            nc.vector.tensor_scalar_max(hfA[:], hfA[:], 0.0)
            nc.vector.scalar_tensor_tensor(
                out=hfA[:], in0=etmp2, scalar=-1.0, in1=hfA[:],
                op0=OP.add, op1=OP.add)

            pooled = ppool.tile([G, C2], F32)
            for t in range(NT):
                oh = work.tile([P, G], F32, tag="oh")
                nc.vector.tensor_tensor(
                    out=oh[:],
                    in0=svap(btf_s, t, [[0, G]]),
                    in1=iof[:], op=OP.is_equal)
                nc.tensor.matmul(
                    pooled[:], lhsT=oh[:],
                    rhs=hfA[:, t * C2:(t + 1) * C2],
                    start=(t == 0), stop=(t == NT - 1))

            # ---- AllReduce partial pooled sums; replicated MLP head ----
            po = const.tile([G, C2], F32)
            nc.vector.tensor_copy(out=po[:], in_=pooled[:])
            nc.sync.dma_start(out=pr_in[:], in_=po[:])
            cc("AllReduce", OP.add, [pr_in[:]], [pr_out[:]])
            sums = const.tile([G, C2], F32)
            nc.sync.dma_start(out=sums[:], in_=pr_out[:])

            cntt = const.tile([G, 1], F32)
            nc.sync.dma_start(out=cntt[:], in_=cnt_d[:])
            wh1 = const.tile([C2, 64], F32)
            nc.sync.dma_start(out=wh1[:], in_=Wh1_d[:])
            bh1 = const.tile([64, 1], F32)
            nc.sync.dma_start(out=bh1[:], in_=bh1_d[:])
            wh2 = const.tile([64, 1], F32)
            nc.sync.dma_start(out=wh2[:], in_=Wh2_d[:])
            bh2 = const.tile([1, 1], F32)
            nc.sync.dma_start(out=bh2[:], in_=bh2_d[:])

            cm = const.tile([G, 1], F32)
            nc.vector.tensor_scalar_max(cm[:], cntt[:], 1.0)
            nc.vector.reciprocal(cm[:], cm[:])
            pooledm = const.tile([G, C2], F32)
            nc.vector.tensor_scalar_mul(pooledm[:], sums[:], cm[:])

            pt = ppool.tile([C2, G], F32, tag="fin")
            nc.tensor.transpose(out=pt[:], in_=pooledm[:],
                                identity=ident[:G, :G])
            pooledT = const.tile([C2, G], F32)
            nc.vector.tensor_copy(out=pooledT[:], in_=pt[:])
            z1 = ppool.tile([64, G], F32, tag="fin2")
            nc.tensor.matmul(z1[:], lhsT=wh1[:], rhs=pooledT[:], start=True,
                             stop=True)
            r1 = const.tile([64, G], F32)
            nc.scalar.activation(r1[:], z1[:], ACT.Relu, bias=bh1[:])
            z2 = ppool.tile([1, G], F32, tag="fin3")
            nc.tensor.matmul(z2[:], lhsT=wh2[:], rhs=r1[:], start=True,
                             stop=True)
            o = const.tile([1, G], F32)
            nc.scalar.activation(o[:], z2[:], ACT.Identity, bias=bh2[:])
            nc.sync.dma_start(out=out_d[:], in_=o[:])
    fix_multiwait(nc)
    return nc


# ---------------------------------------------------------------------------
def make_inmaps(prep, fw):
    in1 = []
    for c in range(NC):
        in1.append(dict(
            xsh=prep["xshs"][c], idx1=prep["idx1s"][c],
            permsc=prep["permscs"][c], btf=prep["btfs"][c],
            iota=prep["iota"], cnt=prep["cnt"], **fw))
    return in1


def kernel(x, edge_index, batch, W1, att_src1, att_dst1, b1,
           W2, att_src2, att_dst2, b2, Wh1, bh1, Wh2, bh2):
    prep = host_prep(x, edge_index, batch)
    fw = fold_weights(W1, att_src1, att_dst1, b1, W2, att_src2, att_dst2, b2,
                      Wh1, bh1, Wh2, bh2)
    nc = build_kernel(prep["Ks"])
    res = run_bass_kernel_spmd(nc, make_inmaps(prep, fw), list(range(NC)))
    return res.results[0]["out"].reshape(G, 1).astype(np.float32)


def _wall_min(fn, n=5):
    import time
    best = 1e9
    for _ in range(n):
        t0 = time.perf_counter()
        fn()
        best = min(best, time.perf_counter() - t0)
    return best


def _null_nc():
    nc = bass.Bass()
    x = nc.declare_dram_parameter("x", [P, 64], F32, isOutput=False)
    y = nc.declare_dram_parameter("y", [P, 64], F32, isOutput=True)
    with ctile.TileContext(nc) as tc:
        with tc.tile_pool(name="sbuf", bufs=1) as pool:
            t = pool.tile([P, 64], F32)
            nc.sync.dma_start(out=t[:], in_=x[:])
            nc.sync.dma_start(out=y[:], in_=t[:])
    fix_multiwait(nc)
    return nc


def timed_run(inputs):
    """Estimate on-device exec ns: warm per-call wall minus null-kernel wall.

    The axon PJRT path exposes no NTFF profiling and the For_i loop repeat
    trick does not compile on this toolchain, so this is an upper-bound
    estimate: warm wall of the single launch minus the warm wall of a trivial
    kernel (same dispatch/tunnel overhead), floored at 0.
    """
    prep = host_prep(inputs["x"], inputs["edge_index"], inputs["batch"])
    fw = fold_weights(inputs["W1"], inputs["att_src1"], inputs["att_dst1"],
                      inputs["b1"], inputs["W2"], inputs["att_src2"],
                      inputs["att_dst2"], inputs["b2"], inputs["Wh1"],
                      inputs["bh1"], inputs["Wh2"], inputs["bh2"])
    cores = list(range(NC))
    in1 = make_inmaps(prep, fw)

    nc0 = _null_nc()
    im0 = [dict(x=np.zeros((P, 64), np.float32)) for _ in range(NC)]
    run_bass_kernel_spmd(nc0, im0, cores)
    t0 = _wall_min(lambda: run_bass_kernel_spmd(nc0, im0, cores), n=8)

    nc1 = build_kernel(prep["Ks"])
    run_bass_kernel_spmd(nc1, in1, cores)
    t1 = _wall_min(lambda: run_bass_kernel_spmd(nc1, in1, cores), n=8)

    d1 = max(t1 - t0, 0.0)
    print(f"null wall {t0*1e3:.1f} ms; launch wall {t1*1e3:.1f} ms")
    print(f"launch exec est {d1*1e6:.0f} us")
    return d1 * 1e9
